# revision 19
# baseline (speedup 1.0000x reference)
"""NeuronPool (moe_routing) Trainium2 kernel.

Expert-parallel over 8 NeuronCores: core c computes neurons [8c, 8c+8) for the
full batch, host concatenates along the neuron axis.

Per-core pipeline (all shapes per core):
  x = [proj | hist_broadcast]  (built on device, stored transposed as 18
      [128,32] bf16 tiles so the batch stays on the PSUM partition dim)
  per neuron n:
      psum1[32,512] = sum_k xT[k].T @ W1[n,k]    (W1 fp8-e3m4 scaled x64 in
          HBM when representable; the 1/64 dequant rides the gelu's ACT
          scale operand for free)
      h1 = gelu(psum1)  [bf16] -> XBAR DMA-transpose -> h1T [128,32] x4
      psum2[32,512] = sum_j h1T[j].T @ W2[n,j]   (W2/W3 bf16 in HBM)
      h2 = gelu(psum2)  -> XBAR transpose -> h2T
      psum3[32,256] = sum_j h2T[j].T @ W3[n,j]
      y = copy(psum3) + row sums; yc = y - mean; ssq(yc)
      B(n), one neuron behind: inv = 1/sqrt(ssq/D+eps); out = yc*inv*(g*mod)
The emission is software-pipelined one neuron deep (GEMM1(n+1) before
tr/GEMM2/GEMM3(n)) so the PE never head-of-line blocks on an ACT gelu.
Biases/beta are all-zero for this model's initializer; the build is
specialized on that (verified at prep time; a general variant with DVE
bias adds is built if any is nonzero).  gamma*mod is pre-broadcast on the
host and DMA'd, so no PE broadcast matmuls remain.
"""
import math
import numpy as np
import ml_dtypes
from contextlib import ExitStack

import concourse.bass as bass
import concourse.tile as tile
from concourse import bacc, mybir
from concourse.bass_utils import run_bass_kernel_spmd

N_CORES = 8
B = 32          # batch
D = 256         # model dim
HIST = 8
HID = 512
N_NEURONS = 64
NPC = N_NEURONS // N_CORES  # 8 neurons per core
IN_DIM = D * (1 + HIST)     # 2304
KC1 = IN_DIM // 128         # 18 contraction chunks for GEMM1
KC2 = HID // 128            # 4 chunks for GEMM2/GEMM3
LN_EPS = 1e-5
FMIN, FMAX = 0.5, 40.0
TICK_INTERVAL = 0.1
W1_SCALE = 64.0             # fp8 pre-scale; 1/W1_SCALE folds into gelu1
FP8_MAX = 15.5              # e3m4 max normal

f32 = mybir.dt.float32
f32r = mybir.dt.float32r
bf16 = mybir.dt.bfloat16
fp8 = mybir.dt.float8e3

_CACHE = {}


def _build_program(zbias, w1_fp8):
    """zbias: b1/b2/b3/beta all zero -> skip bias adds entirely.
    w1_fp8: W1 streams as fp8-e3m4 scaled by W1_SCALE."""
    nc = bacc.Bacc("TRN2", target_bir_lowering=False, debug=False,
                   num_devices=N_CORES)

    emb = nc.dram_tensor("emb", [B, D], f32, kind="ExternalInput").ap()
    wp = nc.dram_tensor("wp", [D, D], f32, kind="ExternalInput").ap()
    bpd = nc.dram_tensor("bpd", [128, 2], f32, kind="ExternalInput").ap()
    histd = nc.dram_tensor("histd", [16, 128], f32, kind="ExternalInput").ap()
    eyed = nc.dram_tensor("eyed", [32, 32], f32, kind="ExternalInput").ap()
    w1d = nc.dram_tensor("w1d", [NPC, 128, KC1, HID],
                         fp8 if w1_fp8 else bf16, kind="ExternalInput").ap()
    w2d = nc.dram_tensor("w2d", [NPC, 128, KC2, HID], bf16,
                         kind="ExternalInput").ap()
    w3d = nc.dram_tensor("w3d", [NPC, 128, KC2, D], bf16,
                         kind="ExternalInput").ap()
    # pre-broadcast per-neuron rows, replicated across the 32 batch
    # partitions on the host: [gm | (b1 b2 b3 bm when not zbias)]
    AUXW = D if zbias else D + HID + HID + D + D
    GM_OFF = 0
    B1_OFF, B2_OFF, B3_OFF, BM_OFF = D, D + HID, D + 2 * HID, 2 * D + 2 * HID
    auxd = nc.dram_tensor("auxd", [B, NPC, AUXW], f32, kind="ExternalInput").ap()
    out = nc.dram_tensor("out", [B, NPC, D], f32, kind="ExternalOutput").ap()

    GELU = mybir.ActivationFunctionType.Gelu
    COPY = mybir.ActivationFunctionType.Copy
    SQUARE = mybir.ActivationFunctionType.Square
    SQRT = mybir.ActivationFunctionType.Sqrt

    with tile.TileContext(nc) as tc, ExitStack() as ctx:
        # SBUF pools
        cst = ctx.enter_context(tc.tile_pool(name="cst", bufs=1))
        xtp = ctx.enter_context(tc.tile_pool(name="xtp", bufs=KC1))
        w1p = ctx.enter_context(tc.tile_pool(name="w1p", bufs=8))
        w23p = ctx.enter_context(tc.tile_pool(name="w23p", bufs=8))
        htp = ctx.enter_context(tc.tile_pool(name="htp", bufs=24))
        hp = ctx.enter_context(tc.tile_pool(name="hp", bufs=4))
        ysp = ctx.enter_context(tc.tile_pool(name="ysp", bufs=4))
        rsp = ctx.enter_context(tc.tile_pool(name="rsp", bufs=4))
        yp = ctx.enter_context(tc.tile_pool(name="yp", bufs=8))
        stp = ctx.enter_context(tc.tile_pool(name="stp", bufs=12))
        # PSUM pools (8 banks: 5 + 2, one spare)
        accp = ctx.enter_context(tc.tile_pool(name="accp", bufs=5, space="PSUM"))
        trp = ctx.enter_context(tc.tile_pool(name="trp", bufs=2, space="PSUM"))

        # ---- weight streaming (issued on the sync HWDGE queue, which is
        # live earliest after kernel start; all transfers are raw copies) ----
        def dma_w1(n, fine=False):
            w1t = []
            if fine:  # neuron 0: 3 smaller pieces so GEMM1(0) starts sooner
                for s in range(3):
                    t = w1p.tile([128, 6, HID], w1d.dtype, tag="w1")
                    nc.sync.dma_start(out=t[:], in_=w1d[n][:, 6 * s:6 * s + 6, :])
                    w1t.append(t)
                return w1t, 6
            for s in range(2):
                t = w1p.tile([128, 9, HID], w1d.dtype, tag="w1")
                nc.sync.dma_start(out=t[:], in_=w1d[n][:, 9 * s:9 * s + 9, :])
                w1t.append(t)
            return w1t, 9

        def dma_w2(n):
            w2t = w23p.tile([128, KC2, HID], bf16, tag="w23")
            nc.sync.dma_start(out=w2t[:], in_=w2d[n])
            return w2t

        def dma_w3(n):
            w3t = w23p.tile([128, KC2, D], bf16, tag="w23")
            nc.sync.dma_start(out=w3t[:], in_=w3d[n])
            return w3t

        # ---- constants ----
        eye = cst.tile([32, 32], f32, tag="eye")
        nc.sync.dma_start(out=eye[:], in_=eyed)
        onesb = cst.tile([128, 32], f32, tag="onesb")
        nc.vector.memset(onesb[:], 1.0)
        epst = cst.tile([B, 1], f32, tag="epst")
        nc.vector.memset(epst[:], LN_EPS)
        bpt = cst.tile([128, 2], f32, tag="bpt")
        nc.sync.dma_start(out=bpt[:], in_=bpd)
        aux = cst.tile([B, NPC, AUXW], f32, tag="aux")
        nc.sync.dma_start(out=aux[:], in_=auxd)

        # first neuron's weights: first on the queue, before the x setup
        w1ts, w2ts, w3ts, h1s = {}, {}, {}, {}
        w1ts[0] = dma_w1(0, fine=True)
        w2ts[0] = dma_w2(0)
        w3ts[0] = dma_w3(0)
        w1ts[1] = dma_w1(1)

        # ---- x setup: xT chunks [128, 32] bf16, k = 0..17 ----
        xT = []

        # proj part: projT = Wp.T @ emb.T + bp, chunks 0..1
        xe = cst.tile([B, D], f32, tag="xe")
        nc.sync.dma_start(out=xe[:], in_=emb)
        wpt = cst.tile([128, 2, D], f32r, tag="wpt")
        nc.gpsimd.dma_start(out=wpt[:], in_=wp.rearrange("(c p) d -> p c d", p=128))
        xeT = []
        for k in range(2):
            pt = trp.tile([128, 32], f32, tag="tr")
            nc.tensor.transpose(pt[:], xe[:, k * 128:(k + 1) * 128], eye[:])
            st = cst.tile([128, 32], f32r, tag=f"xeT{k}")
            nc.vector.tensor_copy(st[:], pt[:])
            xeT.append(st)
        for m in range(2):
            pp = trp.tile([128, 32], f32, tag="tr")
            for k in range(2):
                nc.tensor.matmul(pp[:], wpt[:, k, m * 128:(m + 1) * 128], xeT[k][:],
                                 start=(k == 0), stop=(k == 1))
            xt = xtp.tile([128, 32], bf16, tag="xt")
            nc.vector.tensor_scalar_add(xt[:], pp[:], bpt[:, m:m + 1])
            xT.append(xt)

        # hist part: chunks 2..17 broadcast across batch
        ht = cst.tile([16, 128], f32, tag="ht")
        nc.sync.dma_start(out=ht[:], in_=histd)
        pt = trp.tile([128, 16], f32, tag="tr")
        nc.tensor.transpose(pt[:], ht[:], eye[0:16, 0:16])
        histT = cst.tile([128, 16], f32, tag="histT")
        nc.vector.tensor_copy(histT[:], pt[:])
        for c in range(16):
            xt = xtp.tile([128, 32], bf16, tag="xt")
            nc.vector.tensor_scalar_mul(xt[:], onesb[:], histT[:, c:c + 1])
            xT.append(xt)

        def transpose4(h, width=HID):
            """XBAR DMA-transpose [32,128] bf16 slices -> [128,32] lhsT tiles;
            triggered on the ACT queue right after the gelu that produced h."""
            hT = []
            for j in range(width // 128):
                st = htp.tile([128, 32], bf16, tag="hT")
                nc.scalar.dma_start_transpose(out=st[:], in_=h[:, j * 128:(j + 1) * 128])
                hT.append(st)
            return hT

        def gemm1(n):
            w1t, ch = w1ts[n]
            p1 = accp.tile([B, HID], f32, tag="acc")
            for k in range(KC1):
                nc.tensor.matmul(p1[:], xT[k][:], w1t[k // ch][:, k % ch, :],
                                 start=(k == 0), stop=(k == KC1 - 1))
            h1 = hp.tile([B, HID], bf16, tag="h")
            sc = (1.0 / W1_SCALE) if w1_fp8 else 1.0
            if zbias:
                nc.scalar.activation(h1[:], p1[:], GELU, scale=sc)
            else:
                hb = hp.tile([B, HID], f32, tag="hb")
                nc.vector.tensor_scalar_mul(hb[:], p1[:], sc)
                hc = hp.tile([B, HID], f32, tag="hb")
                nc.vector.tensor_add(hc[:], hb[:], aux[:, n, B1_OFF:B1_OFF + HID])
                nc.scalar.activation(h1[:], hc[:], GELU)
            return h1

        def gemm2(n, h1T):
            w2t = w2ts[n]
            p2 = accp.tile([B, HID], f32, tag="acc")
            for j in range(KC2):
                nc.tensor.matmul(p2[:], h1T[j][:], w2t[:, j, :],
                                 start=(j == 0), stop=(j == KC2 - 1))
            h2 = hp.tile([B, HID], bf16, tag="h")
            if zbias:
                nc.scalar.activation(h2[:], p2[:], GELU)
            else:
                hc = hp.tile([B, HID], f32, tag="hb")
                nc.vector.tensor_add(hc[:], p2[:], aux[:, n, B2_OFF:B2_OFF + HID])
                nc.scalar.activation(h2[:], hc[:], GELU)
            return h2

        ycs = {}
        stats = {}

        def gemm3(n, h2T):
            w3t = w3ts[n]
            p3 = accp.tile([B, D], f32, tag="acc")
            for j in range(KC2):
                nc.tensor.matmul(p3[:], h2T[j][:], w3t[:, j, :],
                                 start=(j == 0), stop=(j == KC2 - 1))

            # center y and accumulate sum(yc^2):
            #   rs = sum(y); yc = y - rs/D; ssq = sum(yc*yc)
            y = yp.tile([B, D], f32, tag="y")
            rs = rsp.tile([B, 1], f32, tag="rs")
            if zbias:
                nc.scalar.activation(y[:], p3[:], COPY, accum_out=rs[:])
            else:
                nc.vector.tensor_add(y[:], p3[:], aux[:, n, B3_OFF:B3_OFF + D])
                yb = yp.tile([B, D], f32, tag="y")
                nc.scalar.activation(yb[:], y[:], COPY, accum_out=rs[:])
                y = yb
            nmu = stp.tile([B, 1], f32, tag="st")
            nc.vector.tensor_scalar_mul(nmu[:], rs[:], -1.0 / D)
            yc = ysp.tile([B, D], f32, tag="ys")
            nc.vector.tensor_scalar_add(yc[:], y[:], nmu[:])
            sqs = yp.tile([B, D], f32, tag="y")
            ssq = stp.tile([B, 1], f32, tag="st")
            nc.scalar.activation(sqs[:], yc[:], SQUARE, accum_out=ssq[:])
            ycs[n] = yc
            stats[n] = ssq

        def emit_B(n):
            yc, ssq = ycs[n], stats[n]
            std = stp.tile([B, 1], f32, tag="st")
            nc.scalar.activation(std[:], ssq[:], SQRT, bias=epst[:], scale=1.0 / D)
            inv = stp.tile([B, 1], f32, tag="st")
            nc.vector.reciprocal(inv[:], std[:])

            yg = yp.tile([B, D], f32, tag="y")
            nc.vector.scalar_tensor_tensor(
                yg[:], yc[:], inv[:], aux[:, n, GM_OFF:GM_OFF + D],
                mybir.AluOpType.mult, mybir.AluOpType.mult)
            if not zbias:
                yo = yp.tile([B, D], f32, tag="y")
                nc.vector.tensor_add(yo[:], yg[:], aux[:, n, BM_OFF:BM_OFF + D])
                yg = yo
            nc.sync.dma_start(out=out[:, n, :], in_=yg[:])

        # ---- software pipeline, one neuron deep ----
        h1s[0] = gemm1(0)
        for n in range(NPC):
            if n + 2 < NPC:
                w1ts[n + 2] = dma_w1(n + 2)
            if n + 1 < NPC:
                w2ts[n + 1] = dma_w2(n + 1)
                w3ts[n + 1] = dma_w3(n + 1)
                h1s[n + 1] = gemm1(n + 1)
            h1T = transpose4(h1s[n])
            h2 = gemm2(n, h1T)
            h2T = transpose4(h2)
            gemm3(n, h2T)
            if n > 0:
                emit_B(n - 1)
        emit_B(NPC - 1)

    nc.compile()
    return nc


def _get_program(zbias, w1_fp8):
    key = (zbias, w1_fp8)
    if key not in _CACHE:
        _CACHE[key] = _build_program(zbias, w1_fp8)
    return _CACHE[key]


def _prep_in_maps(input_embedding, pre_activations, Wp, bp, W1, b1, W2, b2, W3,
                  b3, gamma, beta, tick):
    emb = np.asarray(input_embedding, dtype=np.float32)
    hist = np.asarray(pre_activations, dtype=np.float32)
    Wp = np.asarray(Wp, dtype=np.float32)
    bp = np.asarray(bp, dtype=np.float32)
    W1 = np.asarray(W1, dtype=np.float32)
    b1 = np.asarray(b1, dtype=np.float32)
    W2 = np.asarray(W2, dtype=np.float32)
    b2 = np.asarray(b2, dtype=np.float32)
    W3 = np.asarray(W3, dtype=np.float32)
    b3 = np.asarray(b3, dtype=np.float32)
    gamma = np.asarray(gamma, dtype=np.float32)
    beta = np.asarray(beta, dtype=np.float32)

    zbias = (not b1.any()) and (not b2.any()) and (not b3.any()) \
        and (not beta.any())
    w1_fp8 = float(np.abs(W1).max()) * W1_SCALE <= FP8_MAX

    # oscillator modulation folded into gamma/beta
    i = np.arange(N_NEURONS, dtype=np.float64)
    freq = FMIN * (FMAX / FMIN) ** (i / (N_NEURONS - 1))
    phase = np.mod(i * 2.3571, 2.0 * math.pi)
    t = float(np.asarray(tick)) * TICK_INTERVAL
    mod = (1.0 + 0.5 * np.sin(2.0 * math.pi * freq * t + phase)).astype(np.float32)
    gm = (gamma * mod[:, None]).astype(np.float32)
    bm = (beta * mod[:, None]).astype(np.float32)

    histd = np.ascontiguousarray(hist.reshape(16, 128))
    bpd = np.ascontiguousarray(bp.reshape(2, 128).T)
    eyed = np.eye(32, dtype=np.float32)

    # weight layout: (n, p, k_chunk, hid) so each supertile DMA reads one
    # contiguous run per partition
    W1r = np.ascontiguousarray(
        W1.reshape(N_NEURONS, KC1, 128, HID).transpose(0, 2, 1, 3))
    if w1_fp8:
        W1r = (W1r * W1_SCALE).astype(ml_dtypes.float8_e3m4)
    else:
        W1r = W1r.astype(ml_dtypes.bfloat16)
    W2r = np.ascontiguousarray(
        W2.reshape(N_NEURONS, KC2, 128, HID).transpose(0, 2, 1, 3)).astype(
            ml_dtypes.bfloat16)
    W3r = np.ascontiguousarray(
        W3.reshape(N_NEURONS, KC2, 128, D).transpose(0, 2, 1, 3)).astype(
            ml_dtypes.bfloat16)

    # per-neuron rows pre-broadcast across the batch: [gm | b1 b2 b3 bm]
    if zbias:
        auxn = gm[:, None, :]                                  # (N, 1, D)
        auxn = np.broadcast_to(auxn, (N_NEURONS, B, D))        # (N, B, D)
    else:
        row = np.concatenate([gm, b1, b2, b3, bm], axis=1)
        auxn = np.broadcast_to(row[:, None, :],
                               (N_NEURONS, B, row.shape[1]))
    auxn = np.ascontiguousarray(auxn.transpose(1, 0, 2))       # (B, N, AUXW)

    in_maps = []
    for c in range(N_CORES):
        s = slice(c * NPC, (c + 1) * NPC)
        in_maps.append({
            "emb": emb,
            "wp": Wp,
            "bpd": bpd,
            "histd": histd,
            "eyed": eyed,
            "w1d": W1r[s],
            "w2d": W2r[s],
            "w3d": W3r[s],
            "auxd": np.ascontiguousarray(auxn[:, s, :]),
        })
    return in_maps, zbias, w1_fp8


def run(inputs, trace=False):
    in_maps, zbias, w1_fp8 = _prep_in_maps(**inputs)
    nc = _get_program(zbias, w1_fp8)
    br = run_bass_kernel_spmd(nc, in_maps, core_ids=list(range(N_CORES)),
                              trace=trace)
    out = np.concatenate([r["out"] for r in br.results], axis=1)
    return np.ascontiguousarray(out, dtype=np.float32), br


def kernel(**inputs) -> np.ndarray:
    out, _ = run(inputs, trace=False)
    return out


# revision 21
# speedup vs baseline: 1.7865x; 1.7865x over previous
"""NeuronPool (moe_routing) Trainium2 kernel.

Expert-parallel over 8 NeuronCores: core c computes neurons [8c, 8c+8) for the
full batch, host concatenates along the neuron axis.

Per-core pipeline (all shapes per core):
  x = [proj | hist_broadcast]  (built on device, stored transposed as 18
      [128,32] bf16 tiles so the batch stays on the PSUM partition dim)
  per neuron n:
      psum1[32,512] = sum_k xT[k].T @ W1[n,k]    (W1 fp8-e3m4 scaled x64 in
          HBM when representable; the 1/64 dequant rides the gelu's ACT
          scale operand for free)
      h1 = gelu(psum1)  [bf16] -> XBAR DMA-transpose -> h1T [128,32] x4
      psum2[32,512] = sum_j h1T[j].T @ W2[n,j]   (W2/W3 bf16 in HBM)
      h2 = gelu(psum2)  -> XBAR transpose -> h2T
      psum3[32,256] = sum_j h2T[j].T @ W3[n,j]
      y = copy(psum3) + row sums; yc = y - mean; ssq(yc)
      B(n), one neuron behind: inv = 1/sqrt(ssq/D+eps); out = yc*inv*(g*mod)
The emission is software-pipelined one neuron deep (GEMM1(n+1) before
tr/GEMM2/GEMM3(n)) so the PE never head-of-line blocks on an ACT gelu.
Biases/beta are all-zero for this model's initializer; the build is
specialized on that (verified at prep time; a general variant with DVE
bias adds is built if any is nonzero).  gamma*mod is pre-broadcast on the
host and DMA'd, so no PE broadcast matmuls remain.
"""
import math
import numpy as np
import ml_dtypes
from contextlib import ExitStack

import concourse.bass as bass
import concourse.tile as tile
from concourse import bacc, mybir
from concourse.bass_utils import run_bass_kernel_spmd

N_CORES = 8
B = 32          # batch
D = 256         # model dim
HIST = 8
HID = 512
N_NEURONS = 64
NPC = N_NEURONS // N_CORES  # 8 neurons per core
IN_DIM = D * (1 + HIST)     # 2304
KC1 = IN_DIM // 128         # 18 contraction chunks for GEMM1
KC2 = HID // 128            # 4 chunks for GEMM2/GEMM3
LN_EPS = 1e-5
FMIN, FMAX = 0.5, 40.0
TICK_INTERVAL = 0.1
W1_SCALE = 64.0             # fp8 pre-scale; 1/W1_SCALE folds into gelu1
FP8_MAX = 15.5              # e3m4 max normal

f32 = mybir.dt.float32
f32r = mybir.dt.float32r
bf16 = mybir.dt.bfloat16
fp8 = mybir.dt.float8e3

_CACHE = {}


def _build_program(zbias, w1_fp8):
    """zbias: b1/b2/b3/beta all zero -> skip bias adds entirely.
    w1_fp8: W1 streams as fp8-e3m4 scaled by W1_SCALE."""
    nc = bacc.Bacc("TRN2", target_bir_lowering=False, debug=False,
                   num_devices=N_CORES)

    emb = nc.dram_tensor("emb", [B, D], f32, kind="ExternalInput").ap()
    wp = nc.dram_tensor("wp", [D, D], f32, kind="ExternalInput").ap()
    bpd = nc.dram_tensor("bpd", [128, 2], f32, kind="ExternalInput").ap()
    histd = nc.dram_tensor("histd", [16, 128], f32, kind="ExternalInput").ap()
    eyed = nc.dram_tensor("eyed", [32, 32], f32, kind="ExternalInput").ap()
    w1d = nc.dram_tensor("w1d", [NPC, 128, KC1, HID],
                         fp8 if w1_fp8 else bf16, kind="ExternalInput").ap()
    w2d = nc.dram_tensor("w2d", [NPC, 128, KC2, HID], bf16,
                         kind="ExternalInput").ap()
    w3d = nc.dram_tensor("w3d", [NPC, 128, KC2, D], bf16,
                         kind="ExternalInput").ap()
    # pre-broadcast per-neuron rows, replicated across the 32 batch
    # partitions on the host: [gm | (b1 b2 b3 bm when not zbias)]
    AUXW = D if zbias else D + HID + HID + D + D
    GM_OFF = 0
    B1_OFF, B2_OFF, B3_OFF, BM_OFF = D, D + HID, D + 2 * HID, 2 * D + 2 * HID
    auxd = nc.dram_tensor("auxd", [B, NPC, AUXW], f32, kind="ExternalInput").ap()
    out = nc.dram_tensor("out", [B, NPC, D], f32, kind="ExternalOutput").ap()

    GELU = mybir.ActivationFunctionType.Gelu
    COPY = mybir.ActivationFunctionType.Copy
    SQUARE = mybir.ActivationFunctionType.Square
    SQRT = mybir.ActivationFunctionType.Sqrt

    with tile.TileContext(nc) as tc, ExitStack() as ctx:
        # SBUF pools
        cst = ctx.enter_context(tc.tile_pool(name="cst", bufs=1))
        xtp = ctx.enter_context(tc.tile_pool(name="xtp", bufs=KC1))
        w1p = ctx.enter_context(tc.tile_pool(name="w1p", bufs=8))
        w23p = ctx.enter_context(tc.tile_pool(name="w23p", bufs=8))
        htp = ctx.enter_context(tc.tile_pool(name="htp", bufs=24))
        hp = ctx.enter_context(tc.tile_pool(name="hp", bufs=4))
        ysp = ctx.enter_context(tc.tile_pool(name="ysp", bufs=4))
        rsp = ctx.enter_context(tc.tile_pool(name="rsp", bufs=4))
        yp = ctx.enter_context(tc.tile_pool(name="yp", bufs=8))
        stp = ctx.enter_context(tc.tile_pool(name="stp", bufs=12))
        # PSUM pools (8 banks: 4 + 3, one spare)
        accp = ctx.enter_context(tc.tile_pool(name="accp", bufs=4, space="PSUM"))
        trp = ctx.enter_context(tc.tile_pool(name="trp", bufs=3, space="PSUM"))

        # ---- weight streaming.  The first two neurons are triggered from
        # the sync queue (live earliest after kernel start); the steady
        # stream uses the gpsimd queue so sync stays free for small DMAs ----
        def dma_w1(n, eng=None, fine=False):
            eng = eng or nc.gpsimd
            w1t = []
            if fine:  # neuron 0: 3 smaller pieces so GEMM1(0) starts sooner
                for s in range(3):
                    t = w1p.tile([128, 6, HID], w1d.dtype, tag="w1")
                    eng.dma_start(out=t[:], in_=w1d[n][:, 6 * s:6 * s + 6, :])
                    w1t.append(t)
                return w1t, 6
            for s in range(2):
                t = w1p.tile([128, 9, HID], w1d.dtype, tag="w1")
                eng.dma_start(out=t[:], in_=w1d[n][:, 9 * s:9 * s + 9, :])
                w1t.append(t)
            return w1t, 9

        def dma_w2(n, eng=None):
            eng = eng or nc.gpsimd
            w2t = w23p.tile([128, KC2, HID], bf16, tag="w23")
            eng.dma_start(out=w2t[:], in_=w2d[n])
            return w2t

        def dma_w3(n, eng=None):
            eng = eng or nc.gpsimd
            w3t = w23p.tile([128, KC2, D], bf16, tag="w23")
            eng.dma_start(out=w3t[:], in_=w3d[n])
            return w3t

        # first weight bytes head the sync queue, before the x setup
        w1ts, w2ts, w3ts, h1s = {}, {}, {}, {}
        w1ts[0] = dma_w1(0, eng=nc.sync, fine=True)

        # ---- constants ----
        eye = cst.tile([32, 32], f32, tag="eye")
        nc.sync.dma_start(out=eye[:], in_=eyed)
        onesb = cst.tile([128, 32], f32, tag="onesb")
        nc.vector.memset(onesb[:], 1.0)
        epst = cst.tile([B, 1], f32, tag="epst")
        nc.vector.memset(epst[:], LN_EPS)
        bpt = cst.tile([128, 2], f32, tag="bpt")
        nc.sync.dma_start(out=bpt[:], in_=bpd)
        aux = cst.tile([B, NPC, AUXW], f32, tag="aux")
        nc.sync.dma_start(out=aux[:], in_=auxd)

        w2ts[0] = dma_w2(0, eng=nc.sync)
        w3ts[0] = dma_w3(0, eng=nc.sync)
        w1ts[1] = dma_w1(1, eng=nc.sync)

        # ---- x setup: xT chunks [128, 32] bf16, k = 0..17 ----
        xT = []

        # proj part: projT = Wp.T @ emb.T + bp, chunks 0..1
        xe = cst.tile([B, D], f32, tag="xe")
        nc.sync.dma_start(out=xe[:], in_=emb)
        wpt = cst.tile([128, 2, D], f32r, tag="wpt")
        nc.gpsimd.dma_start(out=wpt[:], in_=wp.rearrange("(c p) d -> p c d", p=128))
        xeT = []
        for k in range(2):
            pt = trp.tile([128, 32], f32, tag="tr")
            nc.tensor.transpose(pt[:], xe[:, k * 128:(k + 1) * 128], eye[:])
            st = cst.tile([128, 32], f32r, tag=f"xeT{k}")
            nc.vector.tensor_copy(st[:], pt[:])
            xeT.append(st)
        for m in range(2):
            pp = trp.tile([128, 32], f32, tag="tr")
            for k in range(2):
                nc.tensor.matmul(pp[:], wpt[:, k, m * 128:(m + 1) * 128], xeT[k][:],
                                 start=(k == 0), stop=(k == 1))
            xt = xtp.tile([128, 32], bf16, tag="xt")
            nc.vector.tensor_scalar_add(xt[:], pp[:], bpt[:, m:m + 1])
            xT.append(xt)

        # hist part: chunks 2..17 broadcast across batch
        ht = cst.tile([16, 128], f32, tag="ht")
        nc.sync.dma_start(out=ht[:], in_=histd)
        pt = trp.tile([128, 16], f32, tag="tr")
        nc.tensor.transpose(pt[:], ht[:], eye[0:16, 0:16])
        histT = cst.tile([128, 16], f32, tag="histT")
        nc.vector.tensor_copy(histT[:], pt[:])
        for c in range(16):
            xt = xtp.tile([128, 32], bf16, tag="xt")
            nc.vector.tensor_scalar_mul(xt[:], onesb[:], histT[:, c:c + 1])
            xT.append(xt)

        eyebf = cst.tile([32, 32], bf16, tag="eyebf")
        nc.vector.tensor_copy(eyebf[:], eye[:])

        def transpose4(h, width=HID):
            """PE transpose (bf16, 1 cycle/row) + DVE copy out of PSUM"""
            hT = []
            for j in range(width // 128):
                pt = trp.tile([128, 32], bf16, tag="tr")
                nc.tensor.transpose(pt[:], h[:, j * 128:(j + 1) * 128], eyebf[:])
                st = htp.tile([128, 32], bf16, tag="hT")
                nc.vector.tensor_copy(st[:], pt[:])
                hT.append(st)
            return hT

        def gemm1(n):
            w1t, ch = w1ts[n]
            p1 = accp.tile([B, HID], f32, tag="acc")
            for k in range(KC1):
                nc.tensor.matmul(p1[:], xT[k][:], w1t[k // ch][:, k % ch, :],
                                 start=(k == 0), stop=(k == KC1 - 1))
            h1 = hp.tile([B, HID], bf16, tag="h")
            sc = (1.0 / W1_SCALE) if w1_fp8 else 1.0
            if zbias:
                nc.scalar.activation(h1[:], p1[:], GELU, scale=sc)
            else:
                hb = hp.tile([B, HID], f32, tag="hb")
                nc.vector.tensor_scalar_mul(hb[:], p1[:], sc)
                hc = hp.tile([B, HID], f32, tag="hb")
                nc.vector.tensor_add(hc[:], hb[:], aux[:, n, B1_OFF:B1_OFF + HID])
                nc.scalar.activation(h1[:], hc[:], GELU)
            return h1

        def gemm2(n, h1T):
            w2t = w2ts[n]
            p2 = accp.tile([B, HID], f32, tag="acc")
            for j in range(KC2):
                nc.tensor.matmul(p2[:], h1T[j][:], w2t[:, j, :],
                                 start=(j == 0), stop=(j == KC2 - 1))
            h2 = hp.tile([B, HID], bf16, tag="h")
            if zbias:
                nc.scalar.activation(h2[:], p2[:], GELU)
            else:
                hc = hp.tile([B, HID], f32, tag="hb")
                nc.vector.tensor_add(hc[:], p2[:], aux[:, n, B2_OFF:B2_OFF + HID])
                nc.scalar.activation(h2[:], hc[:], GELU)
            return h2

        ycs = {}
        stats = {}

        def gemm3(n, h2T):
            w3t = w3ts[n]
            p3 = accp.tile([B, D], f32, tag="acc")
            for j in range(KC2):
                nc.tensor.matmul(p3[:], h2T[j][:], w3t[:, j, :],
                                 start=(j == 0), stop=(j == KC2 - 1))

            # center y and accumulate sum(yc^2):
            #   rs = sum(y); yc = y - rs/D; ssq = sum(yc*yc)
            y = yp.tile([B, D], f32, tag="y")
            rs = rsp.tile([B, 1], f32, tag="rs")
            if zbias:
                nc.scalar.activation(y[:], p3[:], COPY, accum_out=rs[:])
            else:
                nc.vector.tensor_add(y[:], p3[:], aux[:, n, B3_OFF:B3_OFF + D])
                yb = yp.tile([B, D], f32, tag="y")
                nc.scalar.activation(yb[:], y[:], COPY, accum_out=rs[:])
                y = yb
            nmu = stp.tile([B, 1], f32, tag="st")
            nc.vector.tensor_scalar_mul(nmu[:], rs[:], -1.0 / D)
            yc = ysp.tile([B, D], f32, tag="ys")
            nc.vector.tensor_scalar_add(yc[:], y[:], nmu[:])
            sqs = yp.tile([B, D], f32, tag="y")
            ssq = stp.tile([B, 1], f32, tag="st")
            nc.scalar.activation(sqs[:], yc[:], SQUARE, accum_out=ssq[:])
            ycs[n] = yc
            stats[n] = ssq

        def emit_B(n):
            yc, ssq = ycs[n], stats[n]
            std = stp.tile([B, 1], f32, tag="st")
            nc.scalar.activation(std[:], ssq[:], SQRT, bias=epst[:], scale=1.0 / D)
            inv = stp.tile([B, 1], f32, tag="st")
            nc.vector.reciprocal(inv[:], std[:])

            yg = yp.tile([B, D], f32, tag="y")
            nc.vector.scalar_tensor_tensor(
                yg[:], yc[:], inv[:], aux[:, n, GM_OFF:GM_OFF + D],
                mybir.AluOpType.mult, mybir.AluOpType.mult)
            if not zbias:
                yo = yp.tile([B, D], f32, tag="y")
                nc.vector.tensor_add(yo[:], yg[:], aux[:, n, BM_OFF:BM_OFF + D])
                yg = yo
            nc.sync.dma_start(out=out[:, n, :], in_=yg[:])

        # ---- software pipeline, one neuron deep ----
        h1s[0] = gemm1(0)
        for n in range(NPC):
            if n + 2 < NPC:
                w1ts[n + 2] = dma_w1(n + 2)
            if n + 1 < NPC:
                w2ts[n + 1] = dma_w2(n + 1)
                w3ts[n + 1] = dma_w3(n + 1)
                h1s[n + 1] = gemm1(n + 1)
            h1T = transpose4(h1s[n])
            h2 = gemm2(n, h1T)
            h2T = transpose4(h2)
            gemm3(n, h2T)
            if n > 0:
                emit_B(n - 1)
        emit_B(NPC - 1)

    nc.compile()
    return nc


def _get_program(zbias, w1_fp8):
    key = (zbias, w1_fp8)
    if key not in _CACHE:
        _CACHE[key] = _build_program(zbias, w1_fp8)
    return _CACHE[key]


def _prep_in_maps(input_embedding, pre_activations, Wp, bp, W1, b1, W2, b2, W3,
                  b3, gamma, beta, tick):
    emb = np.asarray(input_embedding, dtype=np.float32)
    hist = np.asarray(pre_activations, dtype=np.float32)
    Wp = np.asarray(Wp, dtype=np.float32)
    bp = np.asarray(bp, dtype=np.float32)
    W1 = np.asarray(W1, dtype=np.float32)
    b1 = np.asarray(b1, dtype=np.float32)
    W2 = np.asarray(W2, dtype=np.float32)
    b2 = np.asarray(b2, dtype=np.float32)
    W3 = np.asarray(W3, dtype=np.float32)
    b3 = np.asarray(b3, dtype=np.float32)
    gamma = np.asarray(gamma, dtype=np.float32)
    beta = np.asarray(beta, dtype=np.float32)

    zbias = (not b1.any()) and (not b2.any()) and (not b3.any()) \
        and (not beta.any())
    w1_fp8 = float(np.abs(W1).max()) * W1_SCALE <= FP8_MAX

    # oscillator modulation folded into gamma/beta
    i = np.arange(N_NEURONS, dtype=np.float64)
    freq = FMIN * (FMAX / FMIN) ** (i / (N_NEURONS - 1))
    phase = np.mod(i * 2.3571, 2.0 * math.pi)
    t = float(np.asarray(tick)) * TICK_INTERVAL
    mod = (1.0 + 0.5 * np.sin(2.0 * math.pi * freq * t + phase)).astype(np.float32)
    gm = (gamma * mod[:, None]).astype(np.float32)
    bm = (beta * mod[:, None]).astype(np.float32)

    histd = np.ascontiguousarray(hist.reshape(16, 128))
    bpd = np.ascontiguousarray(bp.reshape(2, 128).T)
    eyed = np.eye(32, dtype=np.float32)

    # weight layout: (n, p, k_chunk, hid) so each supertile DMA reads one
    # contiguous run per partition
    W1r = np.ascontiguousarray(
        W1.reshape(N_NEURONS, KC1, 128, HID).transpose(0, 2, 1, 3))
    if w1_fp8:
        W1r = (W1r * W1_SCALE).astype(ml_dtypes.float8_e3m4)
    else:
        W1r = W1r.astype(ml_dtypes.bfloat16)
    W2r = np.ascontiguousarray(
        W2.reshape(N_NEURONS, KC2, 128, HID).transpose(0, 2, 1, 3)).astype(
            ml_dtypes.bfloat16)
    W3r = np.ascontiguousarray(
        W3.reshape(N_NEURONS, KC2, 128, D).transpose(0, 2, 1, 3)).astype(
            ml_dtypes.bfloat16)

    # per-neuron rows pre-broadcast across the batch: [gm | b1 b2 b3 bm]
    if zbias:
        auxn = gm[:, None, :]                                  # (N, 1, D)
        auxn = np.broadcast_to(auxn, (N_NEURONS, B, D))        # (N, B, D)
    else:
        row = np.concatenate([gm, b1, b2, b3, bm], axis=1)
        auxn = np.broadcast_to(row[:, None, :],
                               (N_NEURONS, B, row.shape[1]))
    auxn = np.ascontiguousarray(auxn.transpose(1, 0, 2))       # (B, N, AUXW)

    in_maps = []
    for c in range(N_CORES):
        s = slice(c * NPC, (c + 1) * NPC)
        in_maps.append({
            "emb": emb,
            "wp": Wp,
            "bpd": bpd,
            "histd": histd,
            "eyed": eyed,
            "w1d": W1r[s],
            "w2d": W2r[s],
            "w3d": W3r[s],
            "auxd": np.ascontiguousarray(auxn[:, s, :]),
        })
    return in_maps, zbias, w1_fp8


def run(inputs, trace=False):
    in_maps, zbias, w1_fp8 = _prep_in_maps(**inputs)
    nc = _get_program(zbias, w1_fp8)
    br = run_bass_kernel_spmd(nc, in_maps, core_ids=list(range(N_CORES)),
                              trace=trace)
    out = np.concatenate([r["out"] for r in br.results], axis=1)
    return np.ascontiguousarray(out, dtype=np.float32), br


def kernel(**inputs) -> np.ndarray:
    out, _ = run(inputs, trace=False)
    return out


# revision 25
# speedup vs baseline: 2.3557x; 1.3186x over previous
"""NeuronPool (moe_routing) Trainium2 kernel.

Expert-parallel over 8 NeuronCores: core c computes neurons [8c, 8c+8) for the
full batch, host concatenates along the neuron axis.

Per-core pipeline (all shapes per core):
  x = [proj | hist_broadcast]  (built on device, stored transposed as 18
      [128,32] bf16 tiles so the batch stays on the PSUM partition dim)
  per neuron n:
      psum1[32,512] = sum_k xT[k].T @ W1[n,k]    (W1 fp8-e3m4 scaled x64 in
          HBM when representable; the 1/64 dequant rides the gelu's ACT
          scale operand for free)
      h1 = gelu(psum1)  [bf16] -> XBAR DMA-transpose -> h1T [128,32] x4
      psum2[32,512] = sum_j h1T[j].T @ W2[n,j]   (W2/W3 bf16 in HBM)
      h2 = gelu(psum2)  -> XBAR transpose -> h2T
      psum3[32,256] = sum_j h2T[j].T @ W3[n,j]
      y = copy(psum3) + row sums; yc = y - mean; ssq(yc)
      B(n), one neuron behind: inv = 1/sqrt(ssq/D+eps); out = yc*inv*(g*mod)
The emission is software-pipelined one neuron deep (GEMM1(n+1) before
tr/GEMM2/GEMM3(n)) so the PE never head-of-line blocks on an ACT gelu.
Biases/beta are all-zero for this model's initializer; the build is
specialized on that (verified at prep time; a general variant with DVE
bias adds is built if any is nonzero).  gamma*mod is pre-broadcast on the
host and DMA'd, so no PE broadcast matmuls remain.
"""
import math
import numpy as np
import ml_dtypes
from contextlib import ExitStack

import concourse.bass as bass
import concourse.tile as tile
from concourse import bacc, mybir
from concourse.bass_utils import run_bass_kernel_spmd

N_CORES = 8
B = 32          # batch
D = 256         # model dim
HIST = 8
HID = 512
N_NEURONS = 64
NPC = N_NEURONS // N_CORES  # 8 neurons per core
IN_DIM = D * (1 + HIST)     # 2304
KC1 = IN_DIM // 128         # 18 contraction chunks for GEMM1
KC2 = HID // 128            # 4 chunks for GEMM2/GEMM3
LN_EPS = 1e-5
FMIN, FMAX = 0.5, 40.0
TICK_INTERVAL = 0.1
W1_SCALE = 64.0             # fp8 pre-scale; 1/W1_SCALE folds into gelu1
FP8_MAX = 15.5              # e3m4 max normal

f32 = mybir.dt.float32
f32r = mybir.dt.float32r
bf16 = mybir.dt.bfloat16
fp8 = mybir.dt.float8e3

_CACHE = {}


def _build_program(zbias, w1_fp8):
    """zbias: b1/b2/b3/beta all zero -> skip bias adds entirely.
    w1_fp8: W1 streams as fp8-e3m4 scaled by W1_SCALE."""
    nc = bacc.Bacc("TRN2", target_bir_lowering=False, debug=False,
                   num_devices=N_CORES)

    emb = nc.dram_tensor("emb", [B, D], f32, kind="ExternalInput").ap()
    wp = nc.dram_tensor("wp", [D, D], f32, kind="ExternalInput").ap()
    bpd = nc.dram_tensor("bpd", [128, 2], f32, kind="ExternalInput").ap()
    histd = nc.dram_tensor("histd", [16, 128], f32, kind="ExternalInput").ap()
    eyed = nc.dram_tensor("eyed", [32, 32], f32, kind="ExternalInput").ap()
    w1d = nc.dram_tensor("w1d", [NPC, 128, KC1, HID],
                         fp8 if w1_fp8 else bf16, kind="ExternalInput").ap()
    w2d = nc.dram_tensor("w2d", [NPC, 128, KC2, HID], bf16,
                         kind="ExternalInput").ap()
    w3d = nc.dram_tensor("w3d", [NPC, 128, KC2, D], bf16,
                         kind="ExternalInput").ap()
    # pre-broadcast per-neuron rows, replicated across the 32 batch
    # partitions on the host: [gm | (b1 b2 b3 bm when not zbias)]
    AUXW = D if zbias else D + HID + HID + D + D
    GM_OFF = 0
    B1_OFF, B2_OFF, B3_OFF, BM_OFF = D, D + HID, D + 2 * HID, 2 * D + 2 * HID
    auxd = nc.dram_tensor("auxd", [B, NPC, AUXW], f32, kind="ExternalInput").ap()
    out = nc.dram_tensor("out", [B, NPC, D], f32, kind="ExternalOutput").ap()

    GELU = mybir.ActivationFunctionType.Gelu
    COPY = mybir.ActivationFunctionType.Copy
    SQUARE = mybir.ActivationFunctionType.Square
    SQRT = mybir.ActivationFunctionType.Sqrt

    with tile.TileContext(nc) as tc, ExitStack() as ctx:
        # SBUF pools
        cst = ctx.enter_context(tc.tile_pool(name="cst", bufs=1))
        xtp = ctx.enter_context(tc.tile_pool(name="xtp", bufs=KC1))
        w1p = ctx.enter_context(tc.tile_pool(name="w1p", bufs=8))
        w23p = ctx.enter_context(tc.tile_pool(name="w23p", bufs=8))
        htp = ctx.enter_context(tc.tile_pool(name="htp", bufs=24))
        hp = ctx.enter_context(tc.tile_pool(name="hp", bufs=4))
        ysp = ctx.enter_context(tc.tile_pool(name="ysp", bufs=4))
        rsp = ctx.enter_context(tc.tile_pool(name="rsp", bufs=4))
        yp = ctx.enter_context(tc.tile_pool(name="yp", bufs=8))
        stp = ctx.enter_context(tc.tile_pool(name="stp", bufs=12))
        # PSUM pools (8 banks: 4 + 3, one spare)
        accp = ctx.enter_context(tc.tile_pool(name="accp", bufs=4, space="PSUM"))
        trp = ctx.enter_context(tc.tile_pool(name="trp", bufs=3, space="PSUM"))

        # ---- weight streaming on the gpsimd queue; small setup DMAs ride
        # the sync queue so neither blocks the other ----
        def dma_w1(n, fine=False):
            w1t = []
            if fine:  # neuron 0: 3 smaller pieces so GEMM1(0) starts sooner
                for s in range(3):
                    t = w1p.tile([128, 6, HID], w1d.dtype, tag="w1")
                    nc.gpsimd.dma_start(out=t[:], in_=w1d[n][:, 6 * s:6 * s + 6, :])
                    w1t.append(t)
                return w1t, 6
            for s in range(2):
                t = w1p.tile([128, 9, HID], w1d.dtype, tag="w1")
                nc.gpsimd.dma_start(out=t[:], in_=w1d[n][:, 9 * s:9 * s + 9, :])
                w1t.append(t)
            return w1t, 9

        def dma_w2(n):
            w2t = w23p.tile([128, KC2, HID], bf16, tag="w23")
            nc.gpsimd.dma_start(out=w2t[:], in_=w2d[n])
            return w2t

        def dma_w3(n):
            w3t = w23p.tile([128, KC2, D], bf16, tag="w23")
            nc.gpsimd.dma_start(out=w3t[:], in_=w3d[n])
            return w3t

        # first weight bytes head the gpsimd queue
        w1ts, w2ts, w3ts, h1s = {}, {}, {}, {}
        w1ts[0] = dma_w1(0, fine=True)

        # ---- constants ----
        eye = cst.tile([32, 32], f32, tag="eye")
        nc.sync.dma_start(out=eye[:], in_=eyed)
        onesb = cst.tile([128, 32], f32, tag="onesb")
        nc.vector.memset(onesb[:], 1.0)
        epst = cst.tile([B, 1], f32, tag="epst")
        nc.vector.memset(epst[:], LN_EPS)
        bpt = cst.tile([128, 2], f32, tag="bpt")
        nc.sync.dma_start(out=bpt[:], in_=bpd)
        aux = cst.tile([B, NPC, AUXW], f32, tag="aux")
        nc.sync.dma_start(out=aux[:], in_=auxd)

        # ---- x setup: xT chunks [128, 32] bf16, k = 0..17 ----
        xT = []

        # proj part: projT = Wp.T @ emb.T + bp, chunks 0..1
        xe = cst.tile([B, D], f32, tag="xe")
        nc.sync.dma_start(out=xe[:], in_=emb)
        wpt = cst.tile([128, 2, D], f32r, tag="wpt")
        nc.gpsimd.dma_start(out=wpt[:], in_=wp.rearrange("(c p) d -> p c d", p=128))
        w2ts[0] = dma_w2(0)
        w3ts[0] = dma_w3(0)
        w1ts[1] = dma_w1(1)
        xeT = []
        for k in range(2):
            pt = trp.tile([128, 32], f32, tag="tr")
            nc.tensor.transpose(pt[:], xe[:, k * 128:(k + 1) * 128], eye[:])
            st = cst.tile([128, 32], f32r, tag=f"xeT{k}")
            nc.vector.tensor_copy(st[:], pt[:])
            xeT.append(st)
        for m in range(2):
            pp = trp.tile([128, 32], f32, tag="tr")
            for k in range(2):
                nc.tensor.matmul(pp[:], wpt[:, k, m * 128:(m + 1) * 128], xeT[k][:],
                                 start=(k == 0), stop=(k == 1))
            xt = xtp.tile([128, 32], bf16, tag="xt")
            nc.vector.tensor_scalar_add(xt[:], pp[:], bpt[:, m:m + 1])
            xT.append(xt)

        # hist part: chunks 2..17 broadcast across batch
        ht = cst.tile([16, 128], f32, tag="ht")
        nc.sync.dma_start(out=ht[:], in_=histd)
        pt = trp.tile([128, 16], f32, tag="tr")
        nc.tensor.transpose(pt[:], ht[:], eye[0:16, 0:16])
        histT = cst.tile([128, 16], f32, tag="histT")
        nc.vector.tensor_copy(histT[:], pt[:])
        for c in range(16):
            xt = xtp.tile([128, 32], bf16, tag="xt")
            nc.vector.tensor_scalar_mul(xt[:], onesb[:], histT[:, c:c + 1])
            xT.append(xt)

        eyebf = cst.tile([32, 32], bf16, tag="eyebf")
        nc.vector.tensor_copy(eyebf[:], eye[:])

        def transpose4(h, width=HID):
            """PE transpose (bf16, 1 cycle/row) + DVE copy out of PSUM"""
            hT = []
            for j in range(width // 128):
                pt = trp.tile([128, 32], bf16, tag="tr")
                nc.tensor.transpose(pt[:], h[:, j * 128:(j + 1) * 128], eyebf[:])
                st = htp.tile([128, 32], bf16, tag="hT")
                nc.vector.tensor_copy(st[:], pt[:])
                hT.append(st)
            return hT

        p1s = {}

        def gemm1_half(n, half):
            # GEMM1 emitted in two halves so its matmuls can interleave with
            # neuron n-1's transposes/GEMM2 and cover the gelu latencies
            w1t, ch = w1ts[n]
            if half == 0:
                p1 = accp.tile([B, HID], f32, tag="acc", name=f"p1_{n}")
                p1s[n] = p1
            p1 = p1s[n]
            ks = range(0, 9) if half == 0 else range(9, KC1)
            for k in ks:
                nc.tensor.matmul(p1[:], xT[k][:], w1t[k // ch][:, k % ch, :],
                                 start=(k == 0), stop=(k == KC1 - 1))
            if half == 0:
                return None
            h1 = hp.tile([B, HID], bf16, tag="h")
            sc = (1.0 / W1_SCALE) if w1_fp8 else 1.0
            if zbias:
                nc.scalar.activation(h1[:], p1[:], GELU, scale=sc)
            else:
                hb = hp.tile([B, HID], f32, tag="hb")
                nc.vector.tensor_scalar_mul(hb[:], p1[:], sc)
                hc = hp.tile([B, HID], f32, tag="hb")
                nc.vector.tensor_add(hc[:], hb[:], aux[:, n, B1_OFF:B1_OFF + HID])
                nc.scalar.activation(h1[:], hc[:], GELU)
            return h1

        def gemm2(n, h1T):
            w2t = w2ts[n]
            p2 = accp.tile([B, HID], f32, tag="acc")
            for j in range(KC2):
                nc.tensor.matmul(p2[:], h1T[j][:], w2t[:, j, :],
                                 start=(j == 0), stop=(j == KC2 - 1))
            h2 = hp.tile([B, HID], bf16, tag="h")
            if zbias:
                nc.scalar.activation(h2[:], p2[:], GELU)
            else:
                hc = hp.tile([B, HID], f32, tag="hb")
                nc.vector.tensor_add(hc[:], p2[:], aux[:, n, B2_OFF:B2_OFF + HID])
                nc.scalar.activation(h2[:], hc[:], GELU)
            return h2

        ycs = {}
        stats = {}

        def gemm3(n, h2T):
            w3t = w3ts[n]
            p3 = accp.tile([B, D], f32, tag="acc")
            for j in range(KC2):
                nc.tensor.matmul(p3[:], h2T[j][:], w3t[:, j, :],
                                 start=(j == 0), stop=(j == KC2 - 1))

            # center y and accumulate sum(yc^2):
            #   rs = sum(y); yc = y - rs/D; ssq = sum(yc*yc)
            y = yp.tile([B, D], f32, tag="y")
            rs = rsp.tile([B, 1], f32, tag="rs")
            if zbias:
                nc.scalar.activation(y[:], p3[:], COPY, accum_out=rs[:])
            else:
                nc.vector.tensor_add(y[:], p3[:], aux[:, n, B3_OFF:B3_OFF + D])
                yb = yp.tile([B, D], f32, tag="y")
                nc.scalar.activation(yb[:], y[:], COPY, accum_out=rs[:])
                y = yb
            nmu = stp.tile([B, 1], f32, tag="st")
            nc.vector.tensor_scalar_mul(nmu[:], rs[:], -1.0 / D)
            yc = ysp.tile([B, D], f32, tag="ys")
            nc.vector.tensor_scalar_add(yc[:], y[:], nmu[:])
            sqs = yp.tile([B, D], f32, tag="y")
            ssq = stp.tile([B, 1], f32, tag="st")
            nc.scalar.activation(sqs[:], yc[:], SQUARE, accum_out=ssq[:])
            ycs[n] = yc
            stats[n] = ssq

        def emit_B(n):
            yc, ssq = ycs[n], stats[n]
            std = stp.tile([B, 1], f32, tag="st")
            nc.scalar.activation(std[:], ssq[:], SQRT, bias=epst[:], scale=1.0 / D)
            inv = stp.tile([B, 1], f32, tag="st")
            nc.vector.reciprocal(inv[:], std[:])

            yg = yp.tile([B, D], f32, tag="y")
            nc.vector.scalar_tensor_tensor(
                yg[:], yc[:], inv[:], aux[:, n, GM_OFF:GM_OFF + D],
                mybir.AluOpType.mult, mybir.AluOpType.mult)
            if not zbias:
                yo = yp.tile([B, D], f32, tag="y")
                nc.vector.tensor_add(yo[:], yg[:], aux[:, n, BM_OFF:BM_OFF + D])
                yg = yo
            nc.sync.dma_start(out=out[:, n, :], in_=yg[:])

        # ---- software pipeline, one neuron deep; GEMM1(n+1)'s two halves
        # bracket GEMM2(n) so the PE is never waiting on a gelu ----
        gemm1_half(0, 0)
        h1s[0] = gemm1_half(0, 1)
        for n in range(NPC):
            if n + 2 < NPC:
                w1ts[n + 2] = dma_w1(n + 2)
            if n + 1 < NPC:
                w2ts[n + 1] = dma_w2(n + 1)
                w3ts[n + 1] = dma_w3(n + 1)
                gemm1_half(n + 1, 0)
            h1T = transpose4(h1s[n])
            h2 = gemm2(n, h1T)
            if n + 1 < NPC:
                h1s[n + 1] = gemm1_half(n + 1, 1)
            h2T = transpose4(h2)
            gemm3(n, h2T)
            if n > 0:
                emit_B(n - 1)
        emit_B(NPC - 1)

    nc.compile()
    return nc


def _get_program(zbias, w1_fp8):
    key = (zbias, w1_fp8)
    if key not in _CACHE:
        _CACHE[key] = _build_program(zbias, w1_fp8)
    return _CACHE[key]


def _prep_in_maps(input_embedding, pre_activations, Wp, bp, W1, b1, W2, b2, W3,
                  b3, gamma, beta, tick):
    emb = np.asarray(input_embedding, dtype=np.float32)
    hist = np.asarray(pre_activations, dtype=np.float32)
    Wp = np.asarray(Wp, dtype=np.float32)
    bp = np.asarray(bp, dtype=np.float32)
    W1 = np.asarray(W1, dtype=np.float32)
    b1 = np.asarray(b1, dtype=np.float32)
    W2 = np.asarray(W2, dtype=np.float32)
    b2 = np.asarray(b2, dtype=np.float32)
    W3 = np.asarray(W3, dtype=np.float32)
    b3 = np.asarray(b3, dtype=np.float32)
    gamma = np.asarray(gamma, dtype=np.float32)
    beta = np.asarray(beta, dtype=np.float32)

    zbias = (not b1.any()) and (not b2.any()) and (not b3.any()) \
        and (not beta.any())
    w1_fp8 = float(np.abs(W1).max()) * W1_SCALE <= FP8_MAX

    # oscillator modulation folded into gamma/beta
    i = np.arange(N_NEURONS, dtype=np.float64)
    freq = FMIN * (FMAX / FMIN) ** (i / (N_NEURONS - 1))
    phase = np.mod(i * 2.3571, 2.0 * math.pi)
    t = float(np.asarray(tick)) * TICK_INTERVAL
    mod = (1.0 + 0.5 * np.sin(2.0 * math.pi * freq * t + phase)).astype(np.float32)
    gm = (gamma * mod[:, None]).astype(np.float32)
    bm = (beta * mod[:, None]).astype(np.float32)

    histd = np.ascontiguousarray(hist.reshape(16, 128))
    bpd = np.ascontiguousarray(bp.reshape(2, 128).T)
    eyed = np.eye(32, dtype=np.float32)

    # weight layout: (n, p, k_chunk, hid) so each supertile DMA reads one
    # contiguous run per partition
    W1r = np.ascontiguousarray(
        W1.reshape(N_NEURONS, KC1, 128, HID).transpose(0, 2, 1, 3))
    if w1_fp8:
        W1r = (W1r * W1_SCALE).astype(ml_dtypes.float8_e3m4)
    else:
        W1r = W1r.astype(ml_dtypes.bfloat16)
    W2r = np.ascontiguousarray(
        W2.reshape(N_NEURONS, KC2, 128, HID).transpose(0, 2, 1, 3)).astype(
            ml_dtypes.bfloat16)
    W3r = np.ascontiguousarray(
        W3.reshape(N_NEURONS, KC2, 128, D).transpose(0, 2, 1, 3)).astype(
            ml_dtypes.bfloat16)

    # per-neuron rows pre-broadcast across the batch: [gm | b1 b2 b3 bm]
    if zbias:
        auxn = gm[:, None, :]                                  # (N, 1, D)
        auxn = np.broadcast_to(auxn, (N_NEURONS, B, D))        # (N, B, D)
    else:
        row = np.concatenate([gm, b1, b2, b3, bm], axis=1)
        auxn = np.broadcast_to(row[:, None, :],
                               (N_NEURONS, B, row.shape[1]))
    auxn = np.ascontiguousarray(auxn.transpose(1, 0, 2))       # (B, N, AUXW)

    in_maps = []
    for c in range(N_CORES):
        s = slice(c * NPC, (c + 1) * NPC)
        in_maps.append({
            "emb": emb,
            "wp": Wp,
            "bpd": bpd,
            "histd": histd,
            "eyed": eyed,
            "w1d": W1r[s],
            "w2d": W2r[s],
            "w3d": W3r[s],
            "auxd": np.ascontiguousarray(auxn[:, s, :]),
        })
    return in_maps, zbias, w1_fp8


def run(inputs, trace=False):
    in_maps, zbias, w1_fp8 = _prep_in_maps(**inputs)
    nc = _get_program(zbias, w1_fp8)
    br = run_bass_kernel_spmd(nc, in_maps, core_ids=list(range(N_CORES)),
                              trace=trace)
    out = np.concatenate([r["out"] for r in br.results], axis=1)
    return np.ascontiguousarray(out, dtype=np.float32), br


def kernel(**inputs) -> np.ndarray:
    out, _ = run(inputs, trace=False)
    return out


# revision 29
# speedup vs baseline: 2.3675x; 1.0050x over previous
"""NeuronPool (moe_routing) Trainium2 kernel.

Expert-parallel over 8 NeuronCores: core c computes neurons [8c, 8c+8) for the
full batch, host concatenates along the neuron axis.

Per-core pipeline (all shapes per core):
  x = [proj | hist_broadcast]  (built on device, stored transposed as 18
      [128,32] bf16 tiles so the batch stays on the PSUM partition dim)
  per neuron n:
      psum1[32,512] = sum_k xT[k].T @ W1[n,k]    (W1 fp8-e3m4 scaled x64 in
          HBM when representable; the 1/64 dequant rides the gelu's ACT
          scale operand for free)
      h1 = gelu(psum1)  [bf16] -> XBAR DMA-transpose -> h1T [128,32] x4
      psum2[32,512] = sum_j h1T[j].T @ W2[n,j]   (W2/W3 bf16 in HBM)
      h2 = gelu(psum2)  -> XBAR transpose -> h2T
      psum3[32,256] = sum_j h2T[j].T @ W3[n,j]
      y = copy(psum3) + row sums; yc = y - mean; ssq(yc)
      B(n), one neuron behind: inv = 1/sqrt(ssq/D+eps); out = yc*inv*(g*mod)
The emission is software-pipelined one neuron deep (GEMM1(n+1) before
tr/GEMM2/GEMM3(n)) so the PE never head-of-line blocks on an ACT gelu.
Biases/beta are all-zero for this model's initializer; the build is
specialized on that (verified at prep time; a general variant with DVE
bias adds is built if any is nonzero).  gamma*mod is pre-broadcast on the
host and DMA'd, so no PE broadcast matmuls remain.
"""
import math
import numpy as np
import ml_dtypes
from contextlib import ExitStack

import concourse.bass as bass
import concourse.tile as tile
from concourse import bacc, mybir
from concourse.bass_utils import run_bass_kernel_spmd

N_CORES = 8
B = 32          # batch
D = 256         # model dim
HIST = 8
HID = 512
N_NEURONS = 64
NPC = N_NEURONS // N_CORES  # 8 neurons per core
IN_DIM = D * (1 + HIST)     # 2304
KC1 = IN_DIM // 128         # 18 contraction chunks for GEMM1
KC2 = HID // 128            # 4 chunks for GEMM2/GEMM3
LN_EPS = 1e-5
FMIN, FMAX = 0.5, 40.0
TICK_INTERVAL = 0.1
W1_SCALE = 64.0             # fp8 pre-scale; 1/W1_SCALE folds into gelu1
FP8_MAX = 15.5              # e3m4 max normal

f32 = mybir.dt.float32
f32r = mybir.dt.float32r
bf16 = mybir.dt.bfloat16
fp8 = mybir.dt.float8e3

_CACHE = {}


def _build_program(zbias, w1_fp8):
    """zbias: b1/b2/b3/beta all zero -> skip bias adds entirely.
    w1_fp8: W1 streams as fp8-e3m4 scaled by W1_SCALE."""
    nc = bacc.Bacc("TRN2", target_bir_lowering=False, debug=False,
                   num_devices=N_CORES)

    emb = nc.dram_tensor("emb", [B, D], f32, kind="ExternalInput").ap()
    wp = nc.dram_tensor("wp", [D, D], f32, kind="ExternalInput").ap()
    bpd = nc.dram_tensor("bpd", [128, 2], f32, kind="ExternalInput").ap()
    histd = nc.dram_tensor("histd", [16, 128], f32, kind="ExternalInput").ap()
    eyed = nc.dram_tensor("eyed", [32, 32], f32, kind="ExternalInput").ap()
    w1d = nc.dram_tensor("w1d", [NPC, 128, KC1, HID],
                         fp8 if w1_fp8 else bf16, kind="ExternalInput").ap()
    w2d = nc.dram_tensor("w2d", [NPC, 128, KC2, HID], bf16,
                         kind="ExternalInput").ap()
    w3d = nc.dram_tensor("w3d", [NPC, 128, KC2, D], bf16,
                         kind="ExternalInput").ap()
    # pre-broadcast per-neuron rows, replicated across the 32 batch
    # partitions on the host: [gm | (b1 b2 b3 bm when not zbias)]
    AUXW = D if zbias else D + HID + HID + D + D
    GM_OFF = 0
    B1_OFF, B2_OFF, B3_OFF, BM_OFF = D, D + HID, D + 2 * HID, 2 * D + 2 * HID
    auxd = nc.dram_tensor("auxd", [B, NPC, AUXW], f32, kind="ExternalInput").ap()
    out = nc.dram_tensor("out", [B, NPC, D], f32, kind="ExternalOutput").ap()

    GELU = mybir.ActivationFunctionType.Gelu
    COPY = mybir.ActivationFunctionType.Copy
    SQUARE = mybir.ActivationFunctionType.Square
    SQRT = mybir.ActivationFunctionType.Sqrt

    with tile.TileContext(nc) as tc, ExitStack() as ctx:
        # SBUF pools
        cst = ctx.enter_context(tc.tile_pool(name="cst", bufs=1))
        xtp = ctx.enter_context(tc.tile_pool(name="xtp", bufs=KC1))
        w1p = ctx.enter_context(tc.tile_pool(name="w1p", bufs=8))
        w23p = ctx.enter_context(tc.tile_pool(name="w23p", bufs=8))
        htp = ctx.enter_context(tc.tile_pool(name="htp", bufs=24))
        hp = ctx.enter_context(tc.tile_pool(name="hp", bufs=4))
        ysp = ctx.enter_context(tc.tile_pool(name="ysp", bufs=NPC))
        rsp = ctx.enter_context(tc.tile_pool(name="rsp", bufs=4))
        yp = ctx.enter_context(tc.tile_pool(name="yp", bufs=8))
        stp = ctx.enter_context(tc.tile_pool(name="stp", bufs=12))
        # PSUM pools (8 banks: 4 + 3, one spare)
        accp = ctx.enter_context(tc.tile_pool(name="accp", bufs=4, space="PSUM"))
        trp = ctx.enter_context(tc.tile_pool(name="trp", bufs=3, space="PSUM"))

        # ---- weight streaming on the gpsimd queue; small setup DMAs ride
        # the sync queue so neither blocks the other ----
        def dma_w1(n, fine=False):
            w1t = []
            if fine:  # neuron 0: 3 smaller pieces so GEMM1(0) starts sooner
                for s in range(3):
                    t = w1p.tile([128, 6, HID], w1d.dtype, tag="w1")
                    nc.gpsimd.dma_start(out=t[:], in_=w1d[n][:, 6 * s:6 * s + 6, :])
                    w1t.append(t)
                return w1t, 6
            for s in range(2):
                t = w1p.tile([128, 9, HID], w1d.dtype, tag="w1")
                nc.gpsimd.dma_start(out=t[:], in_=w1d[n][:, 9 * s:9 * s + 9, :])
                w1t.append(t)
            return w1t, 9

        def dma_w2(n):
            w2t = w23p.tile([128, KC2, HID], bf16, tag="w23")
            nc.gpsimd.dma_start(out=w2t[:], in_=w2d[n])
            return w2t

        def dma_w3(n):
            w3t = w23p.tile([128, KC2, D], bf16, tag="w23")
            nc.gpsimd.dma_start(out=w3t[:], in_=w3d[n])
            return w3t

        # first weight bytes head the gpsimd queue
        w1ts, w2ts, w3ts, h1s = {}, {}, {}, {}
        w1ts[0] = dma_w1(0)
        w2ts[0] = dma_w2(0)
        w3ts[0] = dma_w3(0)

        # ---- constants ----
        eye = cst.tile([32, 32], f32, tag="eye")
        nc.sync.dma_start(out=eye[:], in_=eyed)
        onesb = cst.tile([128, 32], f32, tag="onesb")
        nc.vector.memset(onesb[:], 1.0)
        epst = cst.tile([B, 1], f32, tag="epst")
        nc.vector.memset(epst[:], LN_EPS)

        # ---- x setup.  The x chunks are ordered hist-first (the host rolls
        # W1's k-chunks to match): GEMM1(0)'s first half then depends only on
        # the hist broadcast and w1(0)a, not on the emb/Wp projection chain.
        xT = []

        # hist part: chunks 0..15 broadcast across batch
        ht = cst.tile([16, 128], f32, tag="ht")
        nc.sync.dma_start(out=ht[:], in_=histd)
        pt = trp.tile([128, 16], f32, tag="tr")
        nc.tensor.transpose(pt[:], ht[:], eye[0:16, 0:16])
        histT = cst.tile([128, 16], f32, tag="histT")
        nc.vector.tensor_copy(histT[:], pt[:])
        for c in range(16):
            xt = xtp.tile([128, 32], bf16, tag="xt")
            nc.vector.tensor_scalar_mul(xt[:], onesb[:], histT[:, c:c + 1])
            xT.append(xt)

        eyebf = cst.tile([32, 32], bf16, tag="eyebf")
        nc.vector.tensor_copy(eyebf[:], eye[:])

        def transpose4(h, width=HID):
            """PE transpose (bf16, 1 cycle/row) + DVE copy out of PSUM"""
            hT = []
            for j in range(width // 128):
                pt = trp.tile([128, 32], bf16, tag="tr")
                nc.tensor.transpose(pt[:], h[:, j * 128:(j + 1) * 128], eyebf[:])
                st = htp.tile([128, 32], bf16, tag="hT")
                nc.vector.tensor_copy(st[:], pt[:])
                hT.append(st)
            return hT

        p1s = {}

        def gemm1_half(n, half):
            # GEMM1 emitted in two halves so its matmuls can interleave with
            # neuron n-1's transposes/GEMM2 and cover the gelu latencies
            w1t, ch = w1ts[n]
            if half == 0:
                p1 = accp.tile([B, HID], f32, tag="acc", name=f"p1_{n}")
                p1s[n] = p1
            p1 = p1s[n]
            ks = range(0, 9) if half == 0 else range(9, KC1)
            for k in ks:
                nc.tensor.matmul(p1[:], xT[k][:], w1t[k // ch][:, k % ch, :],
                                 start=(k == 0), stop=(k == KC1 - 1))
            if half == 0:
                return None
            h1 = hp.tile([B, HID], bf16, tag="h")
            sc = (1.0 / W1_SCALE) if w1_fp8 else 1.0
            if zbias:
                nc.scalar.activation(h1[:], p1[:], GELU, scale=sc)
            else:
                hb = hp.tile([B, HID], f32, tag="hb")
                nc.vector.tensor_scalar_mul(hb[:], p1[:], sc)
                hc = hp.tile([B, HID], f32, tag="hb")
                nc.vector.tensor_add(hc[:], hb[:], aux[:, n, B1_OFF:B1_OFF + HID])
                nc.scalar.activation(h1[:], hc[:], GELU)
            return h1

        def gemm2(n, h1T):
            w2t = w2ts[n]
            p2 = accp.tile([B, HID], f32, tag="acc")
            for j in range(KC2):
                nc.tensor.matmul(p2[:], h1T[j][:], w2t[:, j, :],
                                 start=(j == 0), stop=(j == KC2 - 1))
            h2 = hp.tile([B, HID], bf16, tag="h")
            if zbias:
                nc.scalar.activation(h2[:], p2[:], GELU)
            else:
                hc = hp.tile([B, HID], f32, tag="hb")
                nc.vector.tensor_add(hc[:], p2[:], aux[:, n, B2_OFF:B2_OFF + HID])
                nc.scalar.activation(h2[:], hc[:], GELU)
            return h2

        ycs = {}
        ssq_all = cst.tile([B, NPC], f32, tag="ssq_all")

        def gemm3(n, h2T):
            w3t = w3ts[n]
            p3 = accp.tile([B, D], f32, tag="acc")
            for j in range(KC2):
                nc.tensor.matmul(p3[:], h2T[j][:], w3t[:, j, :],
                                 start=(j == 0), stop=(j == KC2 - 1))

            # center y and accumulate sum(yc^2):
            #   rs = sum(y); yc = y - rs/D; ssq = sum(yc*yc)
            y = yp.tile([B, D], f32, tag="y")
            rs = rsp.tile([B, 1], f32, tag="rs")
            if zbias:
                nc.scalar.activation(y[:], p3[:], COPY, accum_out=rs[:])
            else:
                nc.vector.tensor_add(y[:], p3[:], aux[:, n, B3_OFF:B3_OFF + D])
                yb = yp.tile([B, D], f32, tag="y")
                nc.scalar.activation(yb[:], y[:], COPY, accum_out=rs[:])
                y = yb
            nmu = stp.tile([B, 1], f32, tag="st")
            nc.vector.tensor_scalar_mul(nmu[:], rs[:], -1.0 / D)
            yc = ysp.tile([B, D], f32, tag="ys")
            nc.vector.tensor_scalar_add(yc[:], y[:], nmu[:])
            sqs = yp.tile([B, D], f32, tag="y")
            nc.scalar.activation(sqs[:], yc[:], SQUARE,
                                 accum_out=ssq_all[:, n:n + 1])
            ycs[n] = yc

        def emit_B_batch(lo, hi):
            # one SQRT instruction per batch: the sqrt activation table lives
            # in a different ACT table set than gelu, and each swap costs
            # ~1.3us of ACT_TABLE_LOAD
            w = hi - lo
            std = stp.tile([B, w], f32, tag="st")
            nc.scalar.activation(std[:], ssq_all[:, lo:hi], SQRT,
                                 bias=epst[:], scale=1.0 / D)
            inv = stp.tile([B, w], f32, tag="st")
            nc.vector.reciprocal(inv[:], std[:])
            for n in range(lo, hi):
                yg = yp.tile([B, D], f32, tag="y")
                nc.vector.scalar_tensor_tensor(
                    yg[:], ycs[n][:], inv[:, n - lo:n - lo + 1],
                    aux[:, n, GM_OFF:GM_OFF + D],
                    mybir.AluOpType.mult, mybir.AluOpType.mult)
                if not zbias:
                    yo = yp.tile([B, D], f32, tag="y")
                    nc.vector.tensor_add(yo[:], yg[:],
                                         aux[:, n, BM_OFF:BM_OFF + D])
                    yg = yo
                nc.sync.dma_start(out=out[:, n, :], in_=yg[:])

        # ---- prologue: GEMM1(0)'s hist half runs while the emb projection
        # chain (xe DMA -> transpose -> Wp matmul) fills chunks 16/17 ----
        gemm1_half(0, 0)

        xe = cst.tile([B, D], f32, tag="xe")
        nc.sync.dma_start(out=xe[:], in_=emb)
        bpt = cst.tile([128, 2], f32, tag="bpt")
        nc.sync.dma_start(out=bpt[:], in_=bpd)
        aux = cst.tile([B, NPC, AUXW], f32, tag="aux")
        nc.sync.dma_start(out=aux[:], in_=auxd)
        wpt = cst.tile([128, 2, D], f32r, tag="wpt")
        nc.gpsimd.dma_start(out=wpt[:], in_=wp.rearrange("(c p) d -> p c d", p=128))
        w1ts[1] = dma_w1(1)

        xeT = []
        for k in range(2):
            pt = trp.tile([128, 32], f32, tag="tr")
            nc.tensor.transpose(pt[:], xe[:, k * 128:(k + 1) * 128], eye[:])
            st = cst.tile([128, 32], f32r, tag=f"xeT{k}")
            nc.vector.tensor_copy(st[:], pt[:])
            xeT.append(st)
        for m in range(2):
            pp = trp.tile([128, 32], f32, tag="tr")
            for k in range(2):
                nc.tensor.matmul(pp[:], wpt[:, k, m * 128:(m + 1) * 128], xeT[k][:],
                                 start=(k == 0), stop=(k == 1))
            xt = xtp.tile([128, 32], bf16, tag="xt")
            nc.vector.tensor_scalar_add(xt[:], pp[:], bpt[:, m:m + 1])
            xT.append(xt)

        # ---- software pipeline, one neuron deep; GEMM1(n+1)'s two halves
        # bracket GEMM2(n) so the PE is never waiting on a gelu ----
        h1s[0] = gemm1_half(0, 1)
        for n in range(NPC):
            if n + 2 < NPC:
                w1ts[n + 2] = dma_w1(n + 2)
            if n + 1 < NPC:
                w2ts[n + 1] = dma_w2(n + 1)
                w3ts[n + 1] = dma_w3(n + 1)
                gemm1_half(n + 1, 0)
            h1T = transpose4(h1s[n])
            h2 = gemm2(n, h1T)
            if n + 1 < NPC:
                h1s[n + 1] = gemm1_half(n + 1, 1)
            h2T = transpose4(h2)
            gemm3(n, h2T)
            if n == 6:
                emit_B_batch(0, 6)
        emit_B_batch(6, NPC)

    nc.compile()
    return nc


def _get_program(zbias, w1_fp8):
    key = (zbias, w1_fp8)
    if key not in _CACHE:
        _CACHE[key] = _build_program(zbias, w1_fp8)
    return _CACHE[key]


def _prep_in_maps(input_embedding, pre_activations, Wp, bp, W1, b1, W2, b2, W3,
                  b3, gamma, beta, tick):
    emb = np.asarray(input_embedding, dtype=np.float32)
    hist = np.asarray(pre_activations, dtype=np.float32)
    Wp = np.asarray(Wp, dtype=np.float32)
    bp = np.asarray(bp, dtype=np.float32)
    W1 = np.asarray(W1, dtype=np.float32)
    b1 = np.asarray(b1, dtype=np.float32)
    W2 = np.asarray(W2, dtype=np.float32)
    b2 = np.asarray(b2, dtype=np.float32)
    W3 = np.asarray(W3, dtype=np.float32)
    b3 = np.asarray(b3, dtype=np.float32)
    gamma = np.asarray(gamma, dtype=np.float32)
    beta = np.asarray(beta, dtype=np.float32)

    zbias = (not b1.any()) and (not b2.any()) and (not b3.any()) \
        and (not beta.any())
    w1_fp8 = float(np.abs(W1).max()) * W1_SCALE <= FP8_MAX

    # oscillator modulation folded into gamma/beta
    i = np.arange(N_NEURONS, dtype=np.float64)
    freq = FMIN * (FMAX / FMIN) ** (i / (N_NEURONS - 1))
    phase = np.mod(i * 2.3571, 2.0 * math.pi)
    t = float(np.asarray(tick)) * TICK_INTERVAL
    mod = (1.0 + 0.5 * np.sin(2.0 * math.pi * freq * t + phase)).astype(np.float32)
    gm = (gamma * mod[:, None]).astype(np.float32)
    bm = (beta * mod[:, None]).astype(np.float32)

    histd = np.ascontiguousarray(hist.reshape(16, 128))
    bpd = np.ascontiguousarray(bp.reshape(2, 128).T)
    eyed = np.eye(32, dtype=np.float32)

    # weight layout: (n, p, k_chunk, hid) so each supertile DMA reads one
    # contiguous run per partition.  k-chunks are rolled so the hist rows
    # come first, matching the kernel's hist-first xT ordering.
    W1r = np.ascontiguousarray(
        np.roll(W1.reshape(N_NEURONS, KC1, 128, HID), -2, axis=1)
        .transpose(0, 2, 1, 3))
    if w1_fp8:
        W1r = (W1r * W1_SCALE).astype(ml_dtypes.float8_e3m4)
    else:
        W1r = W1r.astype(ml_dtypes.bfloat16)
    W2r = np.ascontiguousarray(
        W2.reshape(N_NEURONS, KC2, 128, HID).transpose(0, 2, 1, 3)).astype(
            ml_dtypes.bfloat16)
    W3r = np.ascontiguousarray(
        W3.reshape(N_NEURONS, KC2, 128, D).transpose(0, 2, 1, 3)).astype(
            ml_dtypes.bfloat16)

    # per-neuron rows pre-broadcast across the batch: [gm | b1 b2 b3 bm]
    if zbias:
        auxn = gm[:, None, :]                                  # (N, 1, D)
        auxn = np.broadcast_to(auxn, (N_NEURONS, B, D))        # (N, B, D)
    else:
        row = np.concatenate([gm, b1, b2, b3, bm], axis=1)
        auxn = np.broadcast_to(row[:, None, :],
                               (N_NEURONS, B, row.shape[1]))
    auxn = np.ascontiguousarray(auxn.transpose(1, 0, 2))       # (B, N, AUXW)

    in_maps = []
    for c in range(N_CORES):
        s = slice(c * NPC, (c + 1) * NPC)
        in_maps.append({
            "emb": emb,
            "wp": Wp,
            "bpd": bpd,
            "histd": histd,
            "eyed": eyed,
            "w1d": W1r[s],
            "w2d": W2r[s],
            "w3d": W3r[s],
            "auxd": np.ascontiguousarray(auxn[:, s, :]),
        })
    return in_maps, zbias, w1_fp8


def run(inputs, trace=False):
    in_maps, zbias, w1_fp8 = _prep_in_maps(**inputs)
    nc = _get_program(zbias, w1_fp8)
    br = run_bass_kernel_spmd(nc, in_maps, core_ids=list(range(N_CORES)),
                              trace=trace)
    out = np.concatenate([r["out"] for r in br.results], axis=1)
    return np.ascontiguousarray(out, dtype=np.float32), br


def kernel(**inputs) -> np.ndarray:
    out, _ = run(inputs, trace=False)
    return out
